# revision 1
# baseline (speedup 1.0000x reference)
"""Tensor-parallel 2-layer decoder for 8 TRN2 NeuronCores (Bass/Tile).

Layout convention on device: activations live TRANSPOSED as [feature, seq]
([DM, S] etc.) so that every matmul in the network — projections, attention
scores, attention-value, MLP, lm_head — maps onto nc.tensor.matmul
(out = lhsT.T @ rhs, contraction on the partition axis) with no on-device
transposes at all.

Sharding (TP-8 per the head/FF column-row scheme):
  - Wq/Wk/Wv column-sharded by head (2 q heads + 1 kv head per core),
    Wo row-sharded; KV cache sharded along the head axis.
  - MLP: Wg/Wu column-sharded (FF/8=1024 per core), Wd row-sharded.
  - lm_head column-sharded (V/8=4000 per core).
  - Residual-stream AllReduces (2 per layer) carry h/8 + partial so the AR
    output IS the next residual stream; they are chunked along seq to
    overlap with compute.

Host does: embedding gather, RoPE of the (input) K cache, all weight
sharding / folding of RMS-norm scales into adjacent weights, bf16 casts,
and constant tables (cos/sin, rotation matrix, causal masks).
"""

import math
import numpy as np
import ml_dtypes

import concourse.bass as bass
import concourse.mybir as mybir
import concourse.tile as tile
from concourse import bacc
from concourse import bass_utils

BF = mybir.dt.bfloat16
F32 = mybir.dt.float32
NPBF = ml_dtypes.bfloat16
AF = mybir.ActivationFunctionType
ALU = mybir.AluOpType


class Cfg:
    def __init__(self, L=2, S=2048, CACHE=2048, DM=2048, FF=8192, V=32000,
                 H=16, HKV=8, D=128, NCORES=8, CS=512):
        self.L, self.S, self.CACHE, self.DM, self.FF, self.V = L, S, CACHE, DM, FF, V
        self.H, self.HKV, self.D, self.NCORES = H, HKV, D, NCORES
        self.CS = CS                      # seq chunk size
        self.CH = S // CS                 # number of chunks
        self.HPC = H // NCORES            # q heads per core
        self.KVP = HKV // NCORES          # kv heads per core (must be 1)
        self.FFS = FF // NCORES           # FF shard
        self.VS = V // NCORES             # vocab shard
        self.KD = DM // 128               # DM k-tiles
        self.FK = self.FFS // 128         # FF shard k-tiles
        self.CT = CACHE // 128            # cache key tiles
        self.ST = S // 128                # seq 128-tiles
        self.NDIAG = CS // 128            # diagonal (masked) new-key tiles/chunk
        self.VCS = 200 if self.VS % 200 == 0 else self.VS // 8  # lm_head n-chunk
        assert self.VS % self.VCS == 0
        self.EPS = 1e-6
        self.ROPE_BASE = 10000.0
        assert self.KVP == 1 and self.HPC == H // NCORES
        assert D == 128


def build_nc(c: Cfg):
    nc = bacc.Bacc("TRN2", target_bir_lowering=False, debug=False,
                   num_devices=c.NCORES)

    # ---------------- DRAM I/O ----------------
    h0 = nc.dram_tensor("h0", [c.DM, c.S], F32, kind="ExternalInput").ap()
    cosq = nc.dram_tensor("cosq", [128, c.S], BF, kind="ExternalInput").ap()
    sinq = nc.dram_tensor("sinq", [128, c.S], BF, kind="ExternalInput").ap()
    rt = nc.dram_tensor("rt", [128, 128], BF, kind="ExternalInput").ap()
    masks = nc.dram_tensor("masks", [c.NDIAG, 128, c.CS], BF, kind="ExternalInput").ap()
    lmw = nc.dram_tensor("lmw", [c.DM, c.VS], BF, kind="ExternalInput").ap()
    logits = nc.dram_tensor("logits", [c.S, c.VS], F32, kind="ExternalOutput").ap()

    wq, wk, wv, wo, wg, wu, wd, ktc, vc = [], [], [], [], [], [], [], [], []
    for l in range(c.L):
        wq.append(nc.dram_tensor(f"wq{l}", [c.DM, c.HPC * c.D], BF, kind="ExternalInput").ap())
        wk.append(nc.dram_tensor(f"wk{l}", [c.DM, c.D], BF, kind="ExternalInput").ap())
        wv.append(nc.dram_tensor(f"wv{l}", [c.DM, c.D], BF, kind="ExternalInput").ap())
        wo.append(nc.dram_tensor(f"wo{l}", [c.HPC * c.D, c.DM], BF, kind="ExternalInput").ap())
        wg.append(nc.dram_tensor(f"wg{l}", [c.DM, c.FFS], BF, kind="ExternalInput").ap())
        wu.append(nc.dram_tensor(f"wu{l}", [c.DM, c.FFS], BF, kind="ExternalInput").ap())
        wd.append(nc.dram_tensor(f"wd{l}", [c.FFS, c.DM], BF, kind="ExternalInput").ap())
        ktc.append(nc.dram_tensor(f"ktc{l}", [c.HPC, 128, c.CACHE], BF, kind="ExternalInput").ap())
        vc.append(nc.dram_tensor(f"vc{l}", [c.HPC, c.CACHE, c.D], BF, kind="ExternalInput").ap())

    inv_n = 1.0 / c.NCORES
    qk_scale = 1.0 / math.sqrt(c.D)

    with tile.TileContext(nc) as tc:
        with (
            tc.tile_pool(name="consts", bufs=1) as consts,
            tc.tile_pool(name="weights", bufs=1) as wpool,
            tc.tile_pool(name="kv", bufs=1) as kvpool,
            tc.tile_pool(name="acts", bufs=1) as hpool,
            tc.tile_pool(name="xn", bufs=2) as xnpool,
            tc.tile_pool(name="small", bufs=2) as small,
            tc.tile_pool(name="str3", bufs=3) as str3,
            tc.tile_pool(name="psA", bufs=2, space="PSUM") as psA,
            tc.tile_pool(name="psB", bufs=2, space="PSUM") as psB,
            tc.tile_pool(name="dram", bufs=1, space="DRAM") as dram,
        ):
            # ---- constants ----
            cos_sb = consts.tile([128, c.S], BF)
            sin_sb = consts.tile([128, c.S], BF)
            rt_sb = consts.tile([128, 128], BF)
            mask_sb = consts.tile([128, c.NDIAG, c.CS], BF)
            ones_sb = consts.tile([128, 1], BF)
            eps_sb = consts.tile([1, 1], F32)
            nc.vector.memset(eps_sb[:], c.EPS)
            nc.sync.dma_start(out=cos_sb[:], in_=cosq[:])
            nc.sync.dma_start(out=sin_sb[:], in_=sinq[:])
            nc.sync.dma_start(out=rt_sb[:], in_=rt[:])
            nc.sync.dma_start(out=mask_sb[:], in_=masks.rearrange("r p n -> p r n"))
            nc.vector.memset(ones_sb[:], 1.0)

            # AR bounce buffers (one pair per layer/phase/chunk)
            arin = {}
            arout = {}
            for l in range(c.L):
                for ph in range(2):
                    for ch in range(c.CH):
                        arin[(l, ph, ch)] = dram.tile(
                            [c.DM, c.CS], F32, tag=f"ari{l}_{ph}_{ch}", name=f"ari{l}_{ph}_{ch}")
                        arout[(l, ph, ch)] = dram.tile(
                            [c.DM, c.CS], F32, tag=f"aro{l}_{ph}_{ch}", name=f"aro{l}_{ph}_{ch}")

            def rms_norm_chunk(h_sb, ch, out_tag):
                """h_sb [128, KD, CS] f32 -> xn [128, KD, CS] bf16 (normalized)."""
                ms_ps = psB.tile([1, c.CS], F32, tag="sum", name=f"ms_{out_tag}")
                for k in range(c.KD):
                    xsq = str3.tile([128, c.CS], BF, tag="xsq", bufs=2, name=f"xsq_{out_tag}_{k}")
                    nc.scalar.activation(out=xsq[:], in_=h_sb[:, k, :], func=AF.Square)
                    nc.tensor.matmul(ms_ps[:], ones_sb[:], xsq[:],
                                     start=(k == 0), stop=(k == c.KD - 1))
                rstd = small.tile([1, c.CS], F32, tag="row", bufs=3, name=f"rstd_{out_tag}")
                nc.scalar.activation(out=rstd[:], in_=ms_ps[:], func=AF.Sqrt,
                                     scale=1.0 / c.DM, bias=eps_sb[:])
                nc.vector.reciprocal(out=rstd[:], in_=rstd[:])
                rb = small.tile([128, c.CS], F32, tag="bcast", bufs=3, name=f"rb_{out_tag}")
                nc.gpsimd.partition_broadcast(rb[:], rstd[:])
                xn = xnpool.tile([128, c.KD, c.CS], BF, tag="xn", name=f"xn_{out_tag}")
                for k in range(c.KD):
                    nc.vector.tensor_tensor(out=xn[:, k, :], in0=h_sb[:, k, :],
                                            in1=rb[:], op=ALU.mult)
                return xn

            def load_h_chunk(src_dram, ch, tag):
                h_sb = hpool.tile([128, c.KD, c.CS], F32, tag="h", name=f"h_{tag}")
                nc.scalar.dma_start(
                    out=h_sb[:],
                    in_=src_dram.rearrange("(k p) n -> p k n", p=128))
                return h_sb

            def rope(p_ps, ch, tag):
                """p_ps [128, CS] f32 PSUM -> bf16 SBUF tile, RoPE applied."""
                p_sb = str3.tile([128, c.CS], BF, tag="prj", bufs=2, name=f"prj_{tag}")
                nc.scalar.copy(out=p_sb[:], in_=p_ps[:])
                rot_ps = psA.tile([128, c.CS], F32, tag="pj", bufs=3, name=f"rot_{tag}")
                nc.tensor.matmul(rot_ps[:], rt_sb[:], p_sb[:], start=True, stop=True)
                cs = cos_sb[:, ch * c.CS:(ch + 1) * c.CS]
                sn = sin_sb[:, ch * c.CS:(ch + 1) * c.CS]
                t1 = small.tile([128, c.CS], F32, tag="t1", name=f"t1_{tag}")
                nc.vector.tensor_tensor(out=t1[:], in0=p_sb[:], in1=cs, op=ALU.mult)
                t2 = small.tile([128, c.CS], F32, tag="t2", name=f"t2_{tag}")
                nc.vector.tensor_tensor(out=t2[:], in0=rot_ps[:], in1=sn, op=ALU.mult)
                out = str3.tile([128, c.CS], BF, tag="rope", bufs=3, name=f"rope_{tag}")
                nc.vector.tensor_tensor(out=out[:], in0=t1[:], in1=t2[:], op=ALU.add)
                return out

            h_src = {ch: h0[:, ch * c.CS:(ch + 1) * c.CS] for ch in range(c.CH)}

            for l in range(c.L):
                # ---- per-layer weights / caches ----
                wq_sb = wpool.tile([128, c.KD, c.HPC * c.D], BF, tag="wq", name=f"wq_sb{l}")
                wk_sb = wpool.tile([128, c.KD, c.D], BF, tag="wk", name=f"wk_sb{l}")
                wv_sb = wpool.tile([128, c.KD, c.D], BF, tag="wv", name=f"wv_sb{l}")
                wo_sb = wpool.tile([128, c.HPC, c.DM], BF, tag="wo", name=f"wo_sb{l}")
                nc.scalar.dma_start(out=wq_sb[:], in_=wq[l].rearrange("(k p) n -> p k n", p=128))
                nc.scalar.dma_start(out=wk_sb[:], in_=wk[l].rearrange("(k p) n -> p k n", p=128))
                nc.scalar.dma_start(out=wv_sb[:], in_=wv[l].rearrange("(k p) n -> p k n", p=128))
                nc.scalar.dma_start(out=wo_sb[:], in_=wo[l].rearrange("(h p) n -> p h n", p=128))
                kc_sb = kvpool.tile([128, c.HPC, c.CACHE], BF, tag="kc", name=f"kc_sb{l}")
                vc_sb = kvpool.tile([128, c.HPC, c.CT, c.D], BF, tag="vc", name=f"vc_sb{l}")
                nc.scalar.dma_start(out=kc_sb[:], in_=ktc[l].rearrange("h p t -> p h t"))
                nc.scalar.dma_start(out=vc_sb[:], in_=vc[l].rearrange("h (t p) d -> p h t d", p=128))

                knew = kvpool.tile([128, c.S], BF, tag="knew", name=f"knew{l}")
                vnew = kvpool.tile([128, c.ST, c.D], BF, tag="vnew", name=f"vnew{l}")

                # =============== PHASE A: attention ===============
                for ch in range(c.CH):
                    h_sb = load_h_chunk(h_src[ch], ch, f"a{l}_{ch}")
                    xn = rms_norm_chunk(h_sb, ch, f"a{l}_{ch}")

                    # qT per head (+rope)
                    qf = []
                    for hh in range(c.HPC):
                        q_ps = psA.tile([128, c.CS], F32, tag="pj", bufs=3, name=f"q_ps{l}_{ch}_{hh}")
                        for k in range(c.KD):
                            nc.tensor.matmul(
                                q_ps[:], wq_sb[:, k, hh * c.D:(hh + 1) * c.D],
                                xn[:, k, :], start=(k == 0), stop=(k == c.KD - 1))
                        qf.append(rope(q_ps, ch, f"q{l}_{ch}_{hh}"))
                    # kT new (+rope) -> knew columns
                    k_ps = psA.tile([128, c.CS], F32, tag="pj", bufs=3, name=f"k_ps{l}_{ch}")
                    for k in range(c.KD):
                        nc.tensor.matmul(k_ps[:], wk_sb[:, k, :], xn[:, k, :],
                                         start=(k == 0), stop=(k == c.KD - 1))
                    kf = rope(k_ps, ch, f"k{l}_{ch}")
                    nc.vector.tensor_copy(out=knew[:, ch * c.CS:(ch + 1) * c.CS], in_=kf[:])
                    # v new -> vnew tiles [s,d]
                    for ss in range(c.CS // 128):
                        st = ch * (c.CS // 128) + ss
                        v_ps = psA.tile([128, c.D], F32, tag="pj", bufs=3, name=f"v_ps{l}_{ch}_{ss}")
                        for k in range(c.KD):
                            nc.tensor.matmul(
                                v_ps[:], xn[:, k, ss * 128:(ss + 1) * 128],
                                wv_sb[:, k, :], start=(k == 0), stop=(k == c.KD - 1))
                        nc.scalar.copy(out=vnew[:, st, :], in_=v_ps[:])

                    # attention per head
                    o_sb = []
                    n_new = ch * c.NDIAG + c.NDIAG  # valid new-key tiles
                    for hh in range(c.HPC):
                        o_ps = psB.tile([128, c.CS], F32, tag="oacc", bufs=1,
                                        name=f"o_ps{l}_{ch}_{hh}")
                        s_ps = psB.tile([1, c.CS], F32, tag="sum", name=f"s_ps{l}_{ch}_{hh}")
                        n_tiles = c.CT + n_new
                        for it in range(n_tiles):
                            if it < c.CT:
                                k_lhs = kc_sb[:, hh, it * 128:(it + 1) * 128]
                                v_lhs = vc_sb[:, hh, it, :]
                                diag_r = -1
                            else:
                                j = it - c.CT
                                k_lhs = knew[:, j * 128:(j + 1) * 128]
                                v_lhs = vnew[:, j, :]
                                diag_r = j - ch * c.NDIAG  # >=0 -> masked tile
                            sc_ps = psA.tile([128, c.CS], F32, tag="sc",
                                             name=f"sc{l}_{ch}_{hh}_{it}")
                            nc.tensor.matmul(sc_ps[:], k_lhs, qf[hh][:],
                                             start=True, stop=True)
                            ex = str3.tile([128, c.CS], BF, tag="exp",
                                           name=f"ex{l}_{ch}_{hh}_{it}")
                            nc.scalar.activation(out=ex[:], in_=sc_ps[:],
                                                 func=AF.Exp, scale=qk_scale)
                            if diag_r >= 0:
                                nc.vector.tensor_tensor(
                                    out=ex[:], in0=ex[:], in1=mask_sb[:, diag_r, :],
                                    op=ALU.mult)
                            nc.tensor.matmul(o_ps[:], v_lhs, ex[:],
                                             start=(it == 0), stop=(it == n_tiles - 1))
                            nc.tensor.matmul(s_ps[:], ones_sb[:], ex[:],
                                             start=(it == 0), stop=(it == n_tiles - 1))
                        # normalize
                        rcp = small.tile([1, c.CS], F32, tag="row", bufs=3, name=f"rcp{l}_{ch}_{hh}")
                        nc.vector.reciprocal(out=rcp[:], in_=s_ps[:])
                        rcb = small.tile([128, c.CS], F32, tag="bcast", bufs=3, name=f"rcb{l}_{ch}_{hh}")
                        nc.gpsimd.partition_broadcast(rcb[:], rcp[:])
                        o_b = str3.tile([128, c.CS], BF, tag="osb", bufs=2, name=f"osb{l}_{ch}_{hh}")
                        nc.vector.tensor_tensor(out=o_b[:], in0=o_ps[:], in1=rcb[:],
                                                op=ALU.mult)
                        o_sb.append(o_b)

                    # Wo (+ h/8 fused) -> AR input
                    for m in range(c.KD):
                        wo_ps = psA.tile([128, c.CS], F32, tag="pj", bufs=3, name=f"wo_ps{l}_{ch}_{m}")
                        for hh in range(c.HPC):
                            nc.tensor.matmul(wo_ps[:], wo_sb[:, hh, m * 128:(m + 1) * 128],
                                             o_sb[hh][:], start=(hh == 0),
                                             stop=(hh == c.HPC - 1))
                        bo = str3.tile([128, c.CS], F32, tag="bo", bufs=2, name=f"bo{l}_{ch}_{m}")
                        nc.vector.scalar_tensor_tensor(
                            out=bo[:], in0=h_sb[:, m, :], scalar=inv_n, in1=wo_ps[:],
                            op0=ALU.mult, op1=ALU.add)
                        nc.sync.dma_start(out=arin[(l, 0, ch)][m * 128:(m + 1) * 128, :],
                                          in_=bo[:])
                    nc.gpsimd.collective_compute(
                        "AllReduce", ALU.add,
                        replica_groups=[list(range(c.NCORES))],
                        ins=[arin[(l, 0, ch)].opt()], outs=[arout[(l, 0, ch)].opt()])

                # =============== PHASE B: MLP ===============
                for ch in range(c.CH):
                    h_sb = load_h_chunk(arout[(l, 0, ch)][:], ch, f"b{l}_{ch}")
                    xn = rms_norm_chunk(h_sb, ch, f"b{l}_{ch}")
                    act = xnpool.tile([128, c.FK, c.CS], BF, tag="act", bufs=1, name=f"act{l}_{ch}")
                    for f in range(c.FK):
                        wg_f = str3.tile([128, c.KD, 128], BF, tag="wgf", bufs=2,
                                         name=f"wgf{l}_{ch}_{f}")
                        wu_f = str3.tile([128, c.KD, 128], BF, tag="wuf", bufs=2,
                                         name=f"wuf{l}_{ch}_{f}")
                        nc.scalar.dma_start(
                            out=wg_f[:], in_=wg[l].rearrange("(k p) n -> p k n", p=128)[
                                :, :, f * 128:(f + 1) * 128])
                        nc.scalar.dma_start(
                            out=wu_f[:], in_=wu[l].rearrange("(k p) n -> p k n", p=128)[
                                :, :, f * 128:(f + 1) * 128])
                        g_ps = psA.tile([128, c.CS], F32, tag="pj", bufs=3, name=f"g_ps{l}_{ch}_{f}")
                        u_ps = psA.tile([128, c.CS], F32, tag="pj", bufs=3, name=f"u_ps{l}_{ch}_{f}")
                        for k in range(c.KD):
                            nc.tensor.matmul(g_ps[:], wg_f[:, k, :], xn[:, k, :],
                                             start=(k == 0), stop=(k == c.KD - 1))
                        for k in range(c.KD):
                            nc.tensor.matmul(u_ps[:], wu_f[:, k, :], xn[:, k, :],
                                             start=(k == 0), stop=(k == c.KD - 1))
                        gs = str3.tile([128, c.CS], BF, tag="gs", bufs=2, name=f"gs{l}_{ch}_{f}")
                        nc.scalar.activation(out=gs[:], in_=g_ps[:], func=AF.Sigmoid)
                        gsg = str3.tile([128, c.CS], BF, tag="gsg", bufs=2, name=f"gsg{l}_{ch}_{f}")
                        nc.vector.tensor_tensor(out=gsg[:], in0=gs[:], in1=g_ps[:],
                                                op=ALU.mult)
                        nc.vector.tensor_tensor(out=act[:, f, :], in0=gsg[:], in1=u_ps[:],
                                                op=ALU.mult)
                    for m in range(c.KD):
                        wd_m = str3.tile([128, c.FK, 128], BF, tag="wdm", bufs=2,
                                         name=f"wdm{l}_{ch}_{m}")
                        nc.scalar.dma_start(
                            out=wd_m[:], in_=wd[l].rearrange("(f p) n -> p f n", p=128)[
                                :, :, m * 128:(m + 1) * 128])
                        d_ps = psA.tile([128, c.CS], F32, tag="pj", bufs=3, name=f"d_ps{l}_{ch}_{m}")
                        for f in range(c.FK):
                            nc.tensor.matmul(d_ps[:], wd_m[:, f, :], act[:, f, :],
                                             start=(f == 0), stop=(f == c.FK - 1))
                        bo = str3.tile([128, c.CS], F32, tag="bo", bufs=2, name=f"bod{l}_{ch}_{m}")
                        nc.vector.scalar_tensor_tensor(
                            out=bo[:], in0=h_sb[:, m, :], scalar=inv_n, in1=d_ps[:],
                            op0=ALU.mult, op1=ALU.add)
                        nc.sync.dma_start(out=arin[(l, 1, ch)][m * 128:(m + 1) * 128, :],
                                          in_=bo[:])
                    nc.gpsimd.collective_compute(
                        "AllReduce", ALU.add,
                        replica_groups=[list(range(c.NCORES))],
                        ins=[arin[(l, 1, ch)].opt()], outs=[arout[(l, 1, ch)].opt()])

                h_src = {ch: arout[(l, 1, ch)][:] for ch in range(c.CH)}

            # =============== final norm + lm_head ===============
            for ch in range(c.CH):
                h_sb = load_h_chunk(h_src[ch], ch, f"f{ch}")
                xn = rms_norm_chunk(h_sb, ch, f"f{ch}")
                nvc = c.VS // c.VCS
                for v in range(nvc):
                    lw = str3.tile([128, c.KD, c.VCS], BF, tag="lw", bufs=2,
                                   name=f"lw{ch}_{v}")
                    nc.scalar.dma_start(
                        out=lw[:], in_=lmw.rearrange("(k p) n -> p k n", p=128)[
                            :, :, v * c.VCS:(v + 1) * c.VCS])
                    for ss in range(c.CS // 128):
                        lm_ps = psA.tile([128, c.VCS], F32, tag="pj", bufs=3, name=f"lm_ps{ch}_{v}_{ss}")
                        for k in range(c.KD):
                            nc.tensor.matmul(lm_ps[:], xn[:, k, ss * 128:(ss + 1) * 128],
                                             lw[:, k, :], start=(k == 0),
                                             stop=(k == c.KD - 1))
                        lo = str3.tile([128, c.VCS], F32, tag="lo", bufs=2, name=f"lo{ch}_{v}_{ss}")
                        nc.scalar.copy(out=lo[:], in_=lm_ps[:])
                        nc.sync.dma_start(
                            out=logits[ch * c.CS + ss * 128: ch * c.CS + (ss + 1) * 128,
                                       v * c.VCS:(v + 1) * c.VCS],
                            in_=lo[:])

    nc.compile()
    return nc


# ------------------------- host side -------------------------

def rope_tables(c: Cfg, pos):
    inv = 1.0 / (c.ROPE_BASE ** (np.arange(0, c.D, 2, dtype=np.float32) / c.D))
    f = pos[:, None].astype(np.float32) * inv[None, :]
    emb = np.concatenate([f, f], -1)              # [T, D]
    return np.cos(emb), np.sin(emb)


def host_prep(c: Cfg, inputs):
    """inputs: full fp32 arrays keyed as in setup_inputs(). Returns in_maps."""
    ids = np.asarray(inputs["input_ids"]).reshape(-1)
    embed = np.asarray(inputs["embed"], dtype=np.float32)
    h0 = embed[ids]                               # [S, DM] fp32 gather
    h0T = np.ascontiguousarray(h0.T)              # [DM, S]

    cos_q, sin_q = rope_tables(c, np.arange(c.CACHE, c.CACHE + c.S))
    cosqT = np.ascontiguousarray(cos_q.T).astype(NPBF)   # [D, S]
    sinqT = np.ascontiguousarray(sin_q.T).astype(NPBF)
    cos_c, sin_c = rope_tables(c, np.arange(c.CACHE))

    # rotation matrix R: rot = R @ x ; lhsT for matmul is R.T
    R = np.zeros((c.D, c.D), np.float32)
    half = c.D // 2
    for i in range(half):
        R[i, i + half] = -1.0
        R[i + half, i] = 1.0
    RT = np.ascontiguousarray(R.T).astype(NPBF)

    # diagonal masks: for r, mask[i, j] = 1 if i + 128*r <= j else 0
    m = np.zeros((c.NDIAG, 128, c.CS), np.float32)
    ii = np.arange(128)[:, None]
    jj = np.arange(c.CS)[None, :]
    for r in range(c.NDIAG):
        m[r] = (ii + 128 * r <= jj)
    masks = m.astype(NPBF)

    ln1 = np.asarray(inputs["ln1"], np.float32)
    ln2 = np.asarray(inputs["ln2"], np.float32)
    fnorm = np.asarray(inputs["final_norm"], np.float32)
    Wq = np.asarray(inputs["Wq"], np.float32)
    Wk = np.asarray(inputs["Wk"], np.float32)
    Wv = np.asarray(inputs["Wv"], np.float32)
    Wo = np.asarray(inputs["Wo"], np.float32)
    Wg = np.asarray(inputs["Wg"], np.float32)
    Wu = np.asarray(inputs["Wu"], np.float32)
    Wd = np.asarray(inputs["Wd"], np.float32)
    lm = np.asarray(inputs["lm_head"], np.float32)
    kc = np.asarray(inputs["k_cache"], np.float32)
    vcache = np.asarray(inputs["v_cache"], np.float32)

    # RoPE the k cache on host (positions 0..CACHE-1), all heads
    rot = np.concatenate([-kc[..., half:], kc[..., :half]], -1)
    kc_roped = kc * cos_c + rot * sin_c           # [L, B, H, CACHE, D]

    in_maps = []
    for core in range(c.NCORES):
        d = {}
        d["h0"] = h0T
        d["cosq"], d["sinq"], d["rt"], d["masks"] = cosqT, sinqT, RT, masks
        d["lmw"] = ((lm * fnorm[:, None])[:, core * c.VS:(core + 1) * c.VS]).astype(NPBF)
        hs = slice(core * c.HPC * c.D, (core + 1) * c.HPC * c.D)
        ks = slice(core * c.D, (core + 1) * c.D)
        fs = slice(core * c.FFS, (core + 1) * c.FFS)
        for l in range(c.L):
            d[f"wq{l}"] = ((Wq[l] * ln1[l][:, None])[:, hs]).astype(NPBF)
            d[f"wk{l}"] = ((Wk[l] * ln1[l][:, None])[:, ks]).astype(NPBF)
            d[f"wv{l}"] = ((Wv[l] * ln1[l][:, None])[:, ks]).astype(NPBF)
            d[f"wo{l}"] = np.ascontiguousarray(Wo[l][hs, :]).astype(NPBF)
            d[f"wg{l}"] = ((Wg[l] * ln2[l][:, None])[:, fs]).astype(NPBF)
            d[f"wu{l}"] = ((Wu[l] * ln2[l][:, None])[:, fs]).astype(NPBF)
            d[f"wd{l}"] = np.ascontiguousarray(Wd[l][fs, :]).astype(NPBF)
            kh = kc_roped[l, 0, core * c.HPC:(core + 1) * c.HPC]   # [HPC, CACHE, D]
            d[f"ktc{l}"] = np.ascontiguousarray(kh.transpose(0, 2, 1)).astype(NPBF)
            d[f"vc{l}"] = np.ascontiguousarray(
                vcache[l, 0, core * c.HPC:(core + 1) * c.HPC]).astype(NPBF)
        in_maps.append(d)
    return in_maps


_NC_CACHE = {}


def get_nc(c: Cfg):
    key = (c.L, c.S, c.DM, c.FF, c.V, c.CS)
    if key not in _NC_CACHE:
        _NC_CACHE[key] = build_nc(c)
    return _NC_CACHE[key]


def kernel(**inputs):
    c = Cfg()
    nc = get_nc(c)
    in_maps = host_prep(c, inputs)
    res = bass_utils.run_bass_kernel_spmd(nc, in_maps, core_ids=list(range(c.NCORES)))
    logits = np.concatenate([res.results[i]["logits"] for i in range(c.NCORES)], axis=1)
    return logits[None].astype(np.float32)


# `kernel(**inputs)` defined above via module-level Cfg/get_nc/host_prep.



# revision 15
# speedup vs baseline: 1.6368x; 1.6368x over previous
"""Tensor-parallel 2-layer decoder for 8 TRN2 NeuronCores (Bass/Tile).

Layout convention on device: activations live TRANSPOSED as [feature, seq]
([DM, S] etc.) so that every matmul in the network — projections, attention
scores, attention-value, MLP, lm_head — maps onto nc.tensor.matmul
(out = lhsT.T @ rhs, contraction on the partition axis) with no on-device
transposes at all.

Sharding (TP-8 per the head/FF column-row scheme):
  - Wq/Wk/Wv column-sharded by head (2 q heads + 1 kv head per core),
    Wo row-sharded; KV cache sharded along the head axis.
  - MLP: Wg/Wu column-sharded (FF/8=1024 per core), Wd row-sharded.
  - lm_head column-sharded (V/8=4000 per core).
  - Residual-stream AllReduces (2 per layer) carry h/8 + partial so the AR
    output IS the next residual stream; they are chunked along seq to
    overlap with compute.

Host does: embedding gather, RoPE of the (input) K cache, all weight
sharding / folding of RMS-norm scales into adjacent weights, bf16 casts,
and constant tables (cos/sin, rotation matrix, causal masks).
"""

import math
import numpy as np
import ml_dtypes

import concourse.bass as bass
import concourse.mybir as mybir
import concourse.tile as tile
from concourse import bacc
from concourse import bass_utils

BF = mybir.dt.bfloat16
F32 = mybir.dt.float32
NPBF = ml_dtypes.bfloat16
AF = mybir.ActivationFunctionType
ALU = mybir.AluOpType


class Cfg:
    def __init__(self, L=2, S=2048, CACHE=2048, DM=2048, FF=8192, V=32000,
                 H=16, HKV=8, D=128, NCORES=8, CS=512,
                 no_coll=False, no_lmhead=False):
        self.no_coll = no_coll            # ablation: skip AllReduces
        self.no_lmhead = no_lmhead        # ablation: skip lm_head matmuls
        self.L, self.S, self.CACHE, self.DM, self.FF, self.V = L, S, CACHE, DM, FF, V
        self.H, self.HKV, self.D, self.NCORES = H, HKV, D, NCORES
        self.CS = CS                      # seq chunk size
        self.CH = S // CS                 # number of chunks
        self.HPC = H // NCORES            # q heads per core
        self.KVP = HKV // NCORES          # kv heads per core (must be 1)
        self.FFS = FF // NCORES           # FF shard
        self.VS = V // NCORES             # vocab shard
        self.KD = DM // 128               # DM k-tiles
        self.FK = self.FFS // 128         # FF shard k-tiles
        self.CT = CACHE // 128            # cache key tiles
        self.ST = S // 128                # seq 128-tiles
        self.NDIAG = CS // 128            # diagonal (masked) new-key tiles/chunk
        self.VCS = 500                    # lm_head n-chunk (PSUM bank <=512 f32)
        assert self.VS % self.VCS == 0
        self.EPS = 1e-6
        self.ROPE_BASE = 10000.0
        assert self.KVP == 1 and self.HPC == H // NCORES
        assert D == 128


def build_nc(c: Cfg):
    nc = bacc.Bacc("TRN2", target_bir_lowering=False, debug=False,
                   num_devices=c.NCORES)

    # ---------------- DRAM I/O ----------------
    h0 = nc.dram_tensor("h0", [c.DM, c.S], BF, kind="ExternalInput").ap()
    tick = nc.dram_tensor("tick", [1, 16], F32, kind="ExternalOutput").ap()
    cosq = nc.dram_tensor("cosq", [128, c.S], BF, kind="ExternalInput").ap()
    sinq = nc.dram_tensor("sinq", [128, c.S], BF, kind="ExternalInput").ap()
    rt = nc.dram_tensor("rt", [128, 128], BF, kind="ExternalInput").ap()
    masks = nc.dram_tensor("masks", [c.NDIAG, 128, c.CS], BF, kind="ExternalInput").ap()
    lmw = nc.dram_tensor("lmw", [c.DM, c.VS], BF, kind="ExternalInput").ap()
    logits = nc.dram_tensor("logits", [c.S, c.VS], F32, kind="ExternalOutput").ap()

    wq, wk, wv, wo, wg, wu, wd, ktc, vc = [], [], [], [], [], [], [], [], []
    for l in range(c.L):
        wq.append(nc.dram_tensor(f"wq{l}", [c.DM, c.HPC * c.D], BF, kind="ExternalInput").ap())
        wk.append(nc.dram_tensor(f"wk{l}", [c.DM, c.D], BF, kind="ExternalInput").ap())
        wv.append(nc.dram_tensor(f"wv{l}", [c.DM, c.D], BF, kind="ExternalInput").ap())
        wo.append(nc.dram_tensor(f"wo{l}", [c.HPC * c.D, c.DM], BF, kind="ExternalInput").ap())
        wg.append(nc.dram_tensor(f"wg{l}", [c.DM, c.FFS], BF, kind="ExternalInput").ap())
        wu.append(nc.dram_tensor(f"wu{l}", [c.DM, c.FFS], BF, kind="ExternalInput").ap())
        wd.append(nc.dram_tensor(f"wd{l}", [c.FFS, c.DM], BF, kind="ExternalInput").ap())
        ktc.append(nc.dram_tensor(f"ktc{l}", [c.HPC, 128, c.CACHE], BF, kind="ExternalInput").ap())
        vc.append(nc.dram_tensor(f"vc{l}", [c.HPC, c.CACHE, c.D], BF, kind="ExternalInput").ap())

    inv_n = 1.0 / c.NCORES
    qk_scale = 1.0 / math.sqrt(c.D)

    with tile.TileContext(nc) as tc:
        with (
            tc.tile_pool(name="consts", bufs=1) as consts,
            tc.tile_pool(name="weights", bufs=1) as wpool,
            tc.tile_pool(name="kv", bufs=1) as kvpool,
            tc.tile_pool(name="acts", bufs=1) as hpool,
            tc.tile_pool(name="xn", bufs=2) as xnpool,
            tc.tile_pool(name="small", bufs=2) as small,
            tc.tile_pool(name="str3", bufs=3) as str3,
            tc.tile_pool(name="psA", bufs=2, space="PSUM") as psA,
            tc.tile_pool(name="psB", bufs=2, space="PSUM") as psB,
            tc.tile_pool(name="dram", bufs=1, space="DRAM") as dram,
        ):
            # ---- constants ----
            cos_sb = consts.tile([128, c.S], BF)
            sin_sb = consts.tile([128, c.S], BF)
            rt_sb = consts.tile([128, 128], BF)
            mask_sb = consts.tile([128, c.NDIAG, c.CS], BF)
            ones_sb = consts.tile([128, 1], BF)
            ones32 = consts.tile([128, 1], F32)
            eps_sb = consts.tile([1, 1], F32)
            tick_sb = consts.tile([1, 16], F32)
            nc.vector.memset(eps_sb[:], c.EPS)
            nc.vector.memset(ones32[:], 1.0)
            nc.vector.memset(tick_sb[:], 1.0)
            nc.sync.dma_start(out=cos_sb[:], in_=cosq[:])
            nc.sync.dma_start(out=sin_sb[:], in_=sinq[:])
            nc.sync.dma_start(out=rt_sb[:], in_=rt[:])
            nc.sync.dma_start(out=mask_sb[:], in_=masks.rearrange("r p n -> p r n"))
            nc.vector.memset(ones_sb[:], 1.0)

            # AR bounce buffers (one pair per layer/phase/chunk)
            arin = {}
            arout = {}
            for l in range(c.L):
                for ph in range(2):
                    for ch in range(c.CH):
                        arin[(l, ph, ch)] = dram.tile(
                            [c.DM, c.CS], BF, tag=f"ari{l}_{ph}_{ch}", name=f"ari{l}_{ph}_{ch}")
                        arout[(l, ph, ch)] = dram.tile(
                            [c.DM, c.CS], BF, tag=f"aro{l}_{ph}_{ch}", name=f"aro{l}_{ph}_{ch}",
                            addr_space="Shared")

            def rms_norm_chunk(h_sb, ch, out_tag):
                """h_sb [128, KD, CS] f32 -> xn [128, KD, CS] bf16 (normalized)."""
                ms_ps = psB.tile([1, c.CS], F32, tag="sum", name=f"ms_{out_tag}")
                for k in range(c.KD):
                    xsq = str3.tile([128, c.CS], BF, tag="xsq", bufs=2, name=f"xsq_{out_tag}_{k}")
                    nc.scalar.activation(out=xsq[:], in_=h_sb[:, k, :], func=AF.Square)
                    nc.tensor.matmul(ms_ps[:], ones_sb[:], xsq[:],
                                     start=(k == 0), stop=(k == c.KD - 1))
                rstd = small.tile([1, c.CS], F32, tag="row", bufs=3, name=f"rstd_{out_tag}")
                nc.scalar.activation(out=rstd[:], in_=ms_ps[:], func=AF.Sqrt,
                                     scale=1.0 / c.DM, bias=eps_sb[:])
                nc.vector.reciprocal(out=rstd[:], in_=rstd[:])
                rb = small.tile([128, c.CS], F32, tag="bcast", bufs=2, name=f"rb_{out_tag}")
                nc.gpsimd.partition_broadcast(rb[:], rstd[:])
                xn = xnpool.tile([128, c.KD, c.CS], BF, tag="xn", name=f"xn_{out_tag}")
                for k in range(c.KD):
                    nc.vector.tensor_tensor(out=xn[:, k, :], in0=h_sb[:, k, :],
                                            in1=rb[:], op=ALU.mult)
                return xn

            def load_h_chunk(src_dram, ch, tag):
                h_sb = hpool.tile([128, c.KD, c.CS], BF, tag="h", name=f"h_{tag}")
                nc.scalar.dma_start(
                    out=h_sb[:],
                    in_=src_dram.rearrange("(k p) n -> p k n", p=128))
                return h_sb

            def rope(p_ps, ch, tag):
                """p_ps [128, CS] f32 PSUM -> bf16 SBUF tile, RoPE applied."""
                p_sb = str3.tile([128, c.CS], BF, tag="prj", bufs=2, name=f"prj_{tag}")
                nc.scalar.copy(out=p_sb[:], in_=p_ps[:])
                rot_ps = psA.tile([128, c.CS], F32, tag="pj", bufs=3, name=f"rot_{tag}")
                nc.tensor.matmul(rot_ps[:], rt_sb[:], p_sb[:], start=True, stop=True)
                cs = cos_sb[:, ch * c.CS:(ch + 1) * c.CS]
                sn = sin_sb[:, ch * c.CS:(ch + 1) * c.CS]
                t1 = small.tile([128, c.CS], BF, tag="t1", name=f"t1_{tag}")
                nc.vector.tensor_tensor(out=t1[:], in0=p_sb[:], in1=cs, op=ALU.mult)
                t2 = small.tile([128, c.CS], BF, tag="t2", name=f"t2_{tag}")
                nc.vector.tensor_tensor(out=t2[:], in0=rot_ps[:], in1=sn, op=ALU.mult)
                out = str3.tile([128, c.CS], BF, tag="rope", bufs=3, name=f"rope_{tag}")
                nc.vector.tensor_tensor(out=out[:], in0=t1[:], in1=t2[:], op=ALU.add)
                return out

            h_src = {ch: h0[:, ch * c.CS:(ch + 1) * c.CS] for ch in range(c.CH)}

            for l in range(c.L):
                # ---- per-layer weights / caches ----
                wq_sb = wpool.tile([128, c.KD, c.HPC * c.D], BF, tag="wq", name=f"wq_sb{l}")
                wk_sb = wpool.tile([128, c.KD, c.D], BF, tag="wk", name=f"wk_sb{l}")
                wv_sb = wpool.tile([128, c.KD, c.D], BF, tag="wv", name=f"wv_sb{l}")
                wo_sb = wpool.tile([128, c.HPC, c.DM], BF, tag="wo", name=f"wo_sb{l}")
                nc.scalar.dma_start(out=wq_sb[:], in_=wq[l].rearrange("(k p) n -> p k n", p=128))
                nc.scalar.dma_start(out=wk_sb[:], in_=wk[l].rearrange("(k p) n -> p k n", p=128))
                nc.scalar.dma_start(out=wv_sb[:], in_=wv[l].rearrange("(k p) n -> p k n", p=128))
                nc.scalar.dma_start(out=wo_sb[:], in_=wo[l].rearrange("(h p) n -> p h n", p=128))
                kc_sb = kvpool.tile([128, c.HPC, c.CACHE], BF, tag="kc", name=f"kc_sb{l}")
                vc_sb = kvpool.tile([128, c.HPC, c.CT, c.D], BF, tag="vc", name=f"vc_sb{l}")
                nc.scalar.dma_start(out=kc_sb[:], in_=ktc[l].rearrange("h p t -> p h t"))
                nc.scalar.dma_start(out=vc_sb[:], in_=vc[l].rearrange("h (t p) d -> p h t d", p=128))

                knew = kvpool.tile([128, c.S], BF, tag="knew", name=f"knew{l}")
                vnew = kvpool.tile([128, c.ST, c.D], BF, tag="vnew", name=f"vnew{l}")

                # =============== PHASE A: attention ===============
                for ch in range(c.CH):
                    h_sb = load_h_chunk(h_src[ch], ch, f"a{l}_{ch}")
                    xn = rms_norm_chunk(h_sb, ch, f"a{l}_{ch}")

                    # qT per head (+rope)
                    qf = []
                    for hh in range(c.HPC):
                        q_ps = psA.tile([128, c.CS], F32, tag="pj", bufs=3, name=f"q_ps{l}_{ch}_{hh}")
                        for k in range(c.KD):
                            nc.tensor.matmul(
                                q_ps[:], wq_sb[:, k, hh * c.D:(hh + 1) * c.D],
                                xn[:, k, :], start=(k == 0), stop=(k == c.KD - 1))
                        qf.append(rope(q_ps, ch, f"q{l}_{ch}_{hh}"))
                    # kT new (+rope) -> knew columns
                    k_ps = psA.tile([128, c.CS], F32, tag="pj", bufs=3, name=f"k_ps{l}_{ch}")
                    for k in range(c.KD):
                        nc.tensor.matmul(k_ps[:], wk_sb[:, k, :], xn[:, k, :],
                                         start=(k == 0), stop=(k == c.KD - 1))
                    kf = rope(k_ps, ch, f"k{l}_{ch}")
                    nc.vector.tensor_copy(out=knew[:, ch * c.CS:(ch + 1) * c.CS], in_=kf[:])
                    # v new -> vnew tiles [s,d]
                    for ss in range(c.CS // 128):
                        st = ch * (c.CS // 128) + ss
                        v_ps = psA.tile([128, c.D], F32, tag="pj", bufs=3, name=f"v_ps{l}_{ch}_{ss}")
                        for k in range(c.KD):
                            nc.tensor.matmul(
                                v_ps[:], xn[:, k, ss * 128:(ss + 1) * 128],
                                wv_sb[:, k, :], start=(k == 0), stop=(k == c.KD - 1))
                        nc.scalar.copy(out=vnew[:, st, :], in_=v_ps[:])

                    # attention per head
                    o_sb = []
                    n_new = ch * c.NDIAG + c.NDIAG  # valid new-key tiles
                    for hh in range(c.HPC):
                        o_ps = psB.tile([128, c.CS], F32, tag="oacc", bufs=1,
                                        name=f"o_ps{l}_{ch}_{hh}")
                        sacc = small.tile([128, c.CS], F32, tag="sacc", bufs=2,
                                          name=f"sacc{l}_{ch}_{hh}")
                        n_tiles = c.CT + n_new
                        for it in range(n_tiles):
                            if it < c.CT:
                                k_lhs = kc_sb[:, hh, it * 128:(it + 1) * 128]
                                v_lhs = vc_sb[:, hh, it, :]
                                diag_r = -1
                            else:
                                j = it - c.CT
                                k_lhs = knew[:, j * 128:(j + 1) * 128]
                                v_lhs = vnew[:, j, :]
                                diag_r = j - ch * c.NDIAG  # >=0 -> masked tile
                            sc_ps = psA.tile([128, c.CS], F32, tag="sc",
                                             name=f"sc{l}_{ch}_{hh}_{it}")
                            nc.tensor.matmul(sc_ps[:], k_lhs, qf[hh][:],
                                             start=True, stop=True)
                            ex = str3.tile([128, c.CS], BF, tag="exp",
                                           name=f"ex{l}_{ch}_{hh}_{it}")
                            nc.scalar.activation(out=ex[:], in_=sc_ps[:],
                                                 func=AF.Exp, scale=qk_scale)
                            if diag_r >= 0:
                                nc.vector.tensor_tensor(
                                    out=ex[:], in0=ex[:], in1=mask_sb[:, diag_r, :],
                                    op=ALU.mult)
                            nc.tensor.matmul(o_ps[:], v_lhs, ex[:],
                                             start=(it == 0), stop=(it == n_tiles - 1))
                            # softmax denominator accumulates on DVE (frees PE)
                            if it == 0:
                                nc.vector.tensor_copy(out=sacc[:], in_=ex[:])
                            else:
                                nc.vector.tensor_tensor(out=sacc[:], in0=sacc[:],
                                                        in1=ex[:], op=ALU.add)
                        s_ps = psB.tile([1, c.CS], F32, tag="sum",
                                        name=f"s_ps{l}_{ch}_{hh}")
                        nc.tensor.matmul(s_ps[:], ones32[:], sacc[:],
                                         start=True, stop=True)
                        # normalize
                        rcp = small.tile([1, c.CS], F32, tag="row", bufs=3, name=f"rcp{l}_{ch}_{hh}")
                        nc.vector.reciprocal(out=rcp[:], in_=s_ps[:])
                        rcb = small.tile([128, c.CS], F32, tag="bcast", bufs=2, name=f"rcb{l}_{ch}_{hh}")
                        nc.gpsimd.partition_broadcast(rcb[:], rcp[:])
                        o_b = str3.tile([128, c.CS], BF, tag="osb", bufs=2, name=f"osb{l}_{ch}_{hh}")
                        nc.vector.tensor_tensor(out=o_b[:], in0=o_ps[:], in1=rcb[:],
                                                op=ALU.mult)
                        o_sb.append(o_b)

                    # Wo (+ h/8 fused) -> AR input
                    for m in range(c.KD):
                        wo_ps = psA.tile([128, c.CS], F32, tag="pj", bufs=3, name=f"wo_ps{l}_{ch}_{m}")
                        for hh in range(c.HPC):
                            nc.tensor.matmul(wo_ps[:], wo_sb[:, hh, m * 128:(m + 1) * 128],
                                             o_sb[hh][:], start=(hh == 0),
                                             stop=(hh == c.HPC - 1))
                        bo = str3.tile([128, c.CS], BF, tag="bo", bufs=2, name=f"bo{l}_{ch}_{m}")
                        nc.vector.scalar_tensor_tensor(
                            out=bo[:], in0=h_sb[:, m, :], scalar=inv_n, in1=wo_ps[:],
                            op0=ALU.mult, op1=ALU.add)
                        nc.sync.dma_start(out=arin[(l, 0, ch)][m * 128:(m + 1) * 128, :],
                                          in_=bo[:])
                    nc.gpsimd.collective_compute(
                        "AllReduce", ALU.add,
                        replica_groups=[list(range(c.NCORES))],
                        ins=[arin[(l, 0, ch)].opt()], outs=[arout[(l, 0, ch)].opt()])

                # =============== PHASE B: MLP ===============
                for ch in range(c.CH):
                    h_sb = load_h_chunk(arout[(l, 0, ch)][:], ch, f"b{l}_{ch}")
                    xn = rms_norm_chunk(h_sb, ch, f"b{l}_{ch}")
                    act = xnpool.tile([128, c.FK, c.CS], BF, tag="act", bufs=1, name=f"act{l}_{ch}")
                    for f in range(c.FK):
                        wg_f = str3.tile([128, c.KD, 128], BF, tag="wgf", bufs=2,
                                         name=f"wgf{l}_{ch}_{f}")
                        wu_f = str3.tile([128, c.KD, 128], BF, tag="wuf", bufs=2,
                                         name=f"wuf{l}_{ch}_{f}")
                        nc.scalar.dma_start(
                            out=wg_f[:], in_=wg[l].rearrange("(k p) n -> p k n", p=128)[
                                :, :, f * 128:(f + 1) * 128])
                        nc.scalar.dma_start(
                            out=wu_f[:], in_=wu[l].rearrange("(k p) n -> p k n", p=128)[
                                :, :, f * 128:(f + 1) * 128])
                        g_ps = psA.tile([128, c.CS], F32, tag="pj", bufs=3, name=f"g_ps{l}_{ch}_{f}")
                        u_ps = psA.tile([128, c.CS], F32, tag="pj", bufs=3, name=f"u_ps{l}_{ch}_{f}")
                        for k in range(c.KD):
                            nc.tensor.matmul(g_ps[:], wg_f[:, k, :], xn[:, k, :],
                                             start=(k == 0), stop=(k == c.KD - 1))
                        for k in range(c.KD):
                            nc.tensor.matmul(u_ps[:], wu_f[:, k, :], xn[:, k, :],
                                             start=(k == 0), stop=(k == c.KD - 1))
                        gs = str3.tile([128, c.CS], BF, tag="gs", bufs=2, name=f"gs{l}_{ch}_{f}")
                        nc.scalar.activation(out=gs[:], in_=g_ps[:], func=AF.Sigmoid)
                        gsg = str3.tile([128, c.CS], BF, tag="gsg", bufs=2, name=f"gsg{l}_{ch}_{f}")
                        nc.vector.tensor_tensor(out=gsg[:], in0=gs[:], in1=g_ps[:],
                                                op=ALU.mult)
                        nc.vector.tensor_tensor(out=act[:, f, :], in0=gsg[:], in1=u_ps[:],
                                                op=ALU.mult)
                    for m in range(c.KD):
                        wd_m = str3.tile([128, c.FK, 128], BF, tag="wdm", bufs=2,
                                         name=f"wdm{l}_{ch}_{m}")
                        nc.scalar.dma_start(
                            out=wd_m[:], in_=wd[l].rearrange("(f p) n -> p f n", p=128)[
                                :, :, m * 128:(m + 1) * 128])
                        d_ps = psA.tile([128, c.CS], F32, tag="pj", bufs=3, name=f"d_ps{l}_{ch}_{m}")
                        for f in range(c.FK):
                            nc.tensor.matmul(d_ps[:], wd_m[:, f, :], act[:, f, :],
                                             start=(f == 0), stop=(f == c.FK - 1))
                        bo = str3.tile([128, c.CS], BF, tag="bo", bufs=2, name=f"bod{l}_{ch}_{m}")
                        nc.vector.scalar_tensor_tensor(
                            out=bo[:], in0=h_sb[:, m, :], scalar=inv_n, in1=d_ps[:],
                            op0=ALU.mult, op1=ALU.add)
                        nc.sync.dma_start(out=arin[(l, 1, ch)][m * 128:(m + 1) * 128, :],
                                          in_=bo[:])
                    nc.gpsimd.collective_compute(
                        "AllReduce", ALU.add,
                        replica_groups=[list(range(c.NCORES))],
                        ins=[arin[(l, 1, ch)].opt()], outs=[arout[(l, 1, ch)].opt()])

                h_src = {ch: arout[(l, 1, ch)][:] for ch in range(c.CH)}

            # =============== final norm + lm_head ===============
            for ch in range(c.CH):
                h_sb = load_h_chunk(h_src[ch], ch, f"f{ch}")
                xn = rms_norm_chunk(h_sb, ch, f"f{ch}")
                nvc = c.VS // c.VCS
                for v in range(nvc):
                    lw = str3.tile([128, c.KD, c.VCS], BF, tag="lw", bufs=2,
                                   name=f"lw{ch}_{v}")
                    nc.scalar.dma_start(
                        out=lw[:], in_=lmw.rearrange("(k p) n -> p k n", p=128)[
                            :, :, v * c.VCS:(v + 1) * c.VCS])
                    for ss in range(c.CS // 128):
                        lm_ps = psA.tile([128, c.VCS], F32, tag="pj", bufs=3, name=f"lm_ps{ch}_{v}_{ss}")
                        for k in range(c.KD):
                            nc.tensor.matmul(lm_ps[:], xn[:, k, ss * 128:(ss + 1) * 128],
                                             lw[:, k, :], start=(k == 0),
                                             stop=(k == c.KD - 1))
                        lo = str3.tile([128, c.VCS], F32, tag="lo", bufs=2, name=f"lo{ch}_{v}_{ss}")
                        nc.scalar.copy(out=lo[:], in_=lm_ps[:])
                        nc.sync.dma_start(
                            out=logits[ch * c.CS + ss * 128: ch * c.CS + (ss + 1) * 128,
                                       v * c.VCS:(v + 1) * c.VCS],
                            in_=lo[:])
            nc.sync.dma_start(out=tick[:], in_=tick_sb[:])

    nc.compile()
    return nc


# ------------------------- host side -------------------------

def rope_tables(c: Cfg, pos):
    inv = 1.0 / (c.ROPE_BASE ** (np.arange(0, c.D, 2, dtype=np.float32) / c.D))
    f = pos[:, None].astype(np.float32) * inv[None, :]
    emb = np.concatenate([f, f], -1)              # [T, D]
    return np.cos(emb), np.sin(emb)


def host_prep(c: Cfg, inputs):
    """inputs: full fp32 arrays keyed as in setup_inputs(). Returns in_maps."""
    ids = np.asarray(inputs["input_ids"]).reshape(-1)
    embed = np.asarray(inputs["embed"], dtype=np.float32)
    h0 = embed[ids]                               # [S, DM] fp32 gather
    h0T = np.ascontiguousarray(h0.T).astype(NPBF)  # [DM, S] bf16

    cos_q, sin_q = rope_tables(c, np.arange(c.CACHE, c.CACHE + c.S))
    cosqT = np.ascontiguousarray(cos_q.T).astype(NPBF)   # [D, S]
    sinqT = np.ascontiguousarray(sin_q.T).astype(NPBF)
    cos_c, sin_c = rope_tables(c, np.arange(c.CACHE))

    # rotation matrix R: rot = R @ x ; lhsT for matmul is R.T
    R = np.zeros((c.D, c.D), np.float32)
    half = c.D // 2
    for i in range(half):
        R[i, i + half] = -1.0
        R[i + half, i] = 1.0
    RT = np.ascontiguousarray(R.T).astype(NPBF)

    # diagonal masks: for r, mask[i, j] = 1 if i + 128*r <= j else 0
    m = np.zeros((c.NDIAG, 128, c.CS), np.float32)
    ii = np.arange(128)[:, None]
    jj = np.arange(c.CS)[None, :]
    for r in range(c.NDIAG):
        m[r] = (ii + 128 * r <= jj)
    masks = m.astype(NPBF)

    ln1 = np.asarray(inputs["ln1"], np.float32)
    ln2 = np.asarray(inputs["ln2"], np.float32)
    fnorm = np.asarray(inputs["final_norm"], np.float32)
    Wq = np.asarray(inputs["Wq"], np.float32)
    Wk = np.asarray(inputs["Wk"], np.float32)
    Wv = np.asarray(inputs["Wv"], np.float32)
    Wo = np.asarray(inputs["Wo"], np.float32)
    Wg = np.asarray(inputs["Wg"], np.float32)
    Wu = np.asarray(inputs["Wu"], np.float32)
    Wd = np.asarray(inputs["Wd"], np.float32)
    lm = np.asarray(inputs["lm_head"], np.float32)
    kc = np.asarray(inputs["k_cache"], np.float32)
    vcache = np.asarray(inputs["v_cache"], np.float32)

    # RoPE the k cache on host (positions 0..CACHE-1), all heads
    rot = np.concatenate([-kc[..., half:], kc[..., :half]], -1)
    kc_roped = kc * cos_c + rot * sin_c           # [L, B, H, CACHE, D]

    in_maps = []
    for core in range(c.NCORES):
        d = {}
        d["h0"] = h0T
        d["cosq"], d["sinq"], d["rt"], d["masks"] = cosqT, sinqT, RT, masks
        d["lmw"] = ((lm * fnorm[:, None])[:, core * c.VS:(core + 1) * c.VS]).astype(NPBF)
        hs = slice(core * c.HPC * c.D, (core + 1) * c.HPC * c.D)
        ks = slice(core * c.D, (core + 1) * c.D)
        fs = slice(core * c.FFS, (core + 1) * c.FFS)
        for l in range(c.L):
            d[f"wq{l}"] = ((Wq[l] * ln1[l][:, None])[:, hs]).astype(NPBF)
            d[f"wk{l}"] = ((Wk[l] * ln1[l][:, None])[:, ks]).astype(NPBF)
            d[f"wv{l}"] = ((Wv[l] * ln1[l][:, None])[:, ks]).astype(NPBF)
            d[f"wo{l}"] = np.ascontiguousarray(Wo[l][hs, :]).astype(NPBF)
            d[f"wg{l}"] = ((Wg[l] * ln2[l][:, None])[:, fs]).astype(NPBF)
            d[f"wu{l}"] = ((Wu[l] * ln2[l][:, None])[:, fs]).astype(NPBF)
            d[f"wd{l}"] = np.ascontiguousarray(Wd[l][fs, :]).astype(NPBF)
            kh = kc_roped[l, 0, core * c.HPC:(core + 1) * c.HPC]   # [HPC, CACHE, D]
            d[f"ktc{l}"] = np.ascontiguousarray(kh.transpose(0, 2, 1)).astype(NPBF)
            d[f"vc{l}"] = np.ascontiguousarray(
                vcache[l, 0, core * c.HPC:(core + 1) * c.HPC]).astype(NPBF)
        in_maps.append(d)
    return in_maps


_NC_CACHE = {}


def get_nc(c: Cfg):
    key = (c.L, c.S, c.DM, c.FF, c.V, c.CS)
    if key not in _NC_CACHE:
        _NC_CACHE[key] = build_nc(c)
    return _NC_CACHE[key]


def kernel(**inputs):
    c = Cfg()
    nc = get_nc(c)
    in_maps = host_prep(c, inputs)
    res = bass_utils.run_bass_kernel_spmd(nc, in_maps, core_ids=list(range(c.NCORES)))
    logits = np.concatenate([res.results[i]["logits"] for i in range(c.NCORES)], axis=1)
    return logits[None].astype(np.float32)


# `kernel(**inputs)` defined above via module-level Cfg/get_nc/host_prep.



# revision 23
# speedup vs baseline: 2.3589x; 1.4412x over previous
"""Tensor-parallel 2-layer decoder for 8 TRN2 NeuronCores (Bass/Tile).

Layout convention on device: activations live TRANSPOSED as [feature, seq]
([DM, S] etc.) so that every matmul in the network — projections, attention
scores, attention-value, MLP, lm_head — maps onto nc.tensor.matmul
(out = lhsT.T @ rhs, contraction on the partition axis) with no on-device
transposes at all.

Sharding (TP-8 per the head/FF column-row scheme):
  - Wq/Wk/Wv column-sharded by head (2 q heads + 1 kv head per core),
    Wo row-sharded; KV cache sharded along the head axis.
  - MLP: Wg/Wu column-sharded (FF/8=1024 per core), Wd row-sharded.
  - lm_head column-sharded (V/8=4000 per core).
  - Residual-stream AllReduces (2 per layer) carry h/8 + partial so the AR
    output IS the next residual stream; they are chunked along seq to
    overlap with compute.

Host does: embedding gather, RoPE of the (input) K cache, all weight
sharding / folding of RMS-norm scales into adjacent weights, bf16 casts,
and constant tables (cos/sin, rotation matrix, causal masks).
"""

import math
import numpy as np
import ml_dtypes

import concourse.bass as bass
import concourse.mybir as mybir
import concourse.tile as tile
from concourse import bacc
from concourse import bass_utils

BF = mybir.dt.bfloat16
F32 = mybir.dt.float32
NPBF = ml_dtypes.bfloat16
AF = mybir.ActivationFunctionType
ALU = mybir.AluOpType


class Cfg:
    def __init__(self, L=2, S=2048, CACHE=2048, DM=2048, FF=8192, V=32000,
                 H=16, HKV=8, D=128, NCORES=8, CS=512,
                 no_coll=False, no_lmhead=False):
        self.no_coll = no_coll            # ablation: skip AllReduces
        self.no_lmhead = no_lmhead        # ablation: skip lm_head matmuls
        self.L, self.S, self.CACHE, self.DM, self.FF, self.V = L, S, CACHE, DM, FF, V
        self.H, self.HKV, self.D, self.NCORES = H, HKV, D, NCORES
        self.CS = CS                      # seq chunk size
        self.CH = S // CS                 # number of chunks
        self.HPC = H // NCORES            # q heads per core
        self.KVP = HKV // NCORES          # kv heads per core (must be 1)
        self.FFS = FF // NCORES           # FF shard
        self.VS = V // NCORES             # vocab shard
        self.KD = DM // 128               # DM k-tiles
        self.FK = self.FFS // 128         # FF shard k-tiles
        self.CT = CACHE // 128            # cache key tiles
        self.ST = S // 128                # seq 128-tiles
        self.NDIAG = CS // 128            # diagonal (masked) new-key tiles/chunk
        self.VCS = 500                    # lm_head n-chunk (PSUM bank <=512 f32)
        assert self.VS % self.VCS == 0
        self.EPS = 1e-6
        self.ROPE_BASE = 10000.0
        assert self.KVP == 1 and self.HPC == H // NCORES
        assert D == 128


def build_nc(c: Cfg):
    nc = bacc.Bacc("TRN2", target_bir_lowering=False, debug=False,
                   num_devices=c.NCORES)

    # ---------------- DRAM I/O ----------------
    h0 = nc.dram_tensor("h0", [c.DM, c.S], BF, kind="ExternalInput").ap()
    tick = nc.dram_tensor("tick", [1, 16], F32, kind="ExternalOutput").ap()
    cosq = nc.dram_tensor("cosq", [128, c.S], BF, kind="ExternalInput").ap()
    sinq = nc.dram_tensor("sinq", [128, c.S], BF, kind="ExternalInput").ap()
    rt = nc.dram_tensor("rt", [128, 128], BF, kind="ExternalInput").ap()
    masks = nc.dram_tensor("masks", [c.NDIAG, 128, c.CS], BF, kind="ExternalInput").ap()
    lmw = nc.dram_tensor("lmw", [c.DM, c.VS], BF, kind="ExternalInput").ap()
    logits = nc.dram_tensor("logits", [c.S, c.VS], F32, kind="ExternalOutput").ap()

    wq, wk, wv, wo, wg, wu, wd, ktc, vc = [], [], [], [], [], [], [], [], []
    for l in range(c.L):
        wq.append(nc.dram_tensor(f"wq{l}", [c.DM, c.HPC * c.D], BF, kind="ExternalInput").ap())
        wk.append(nc.dram_tensor(f"wk{l}", [c.DM, c.D], BF, kind="ExternalInput").ap())
        wv.append(nc.dram_tensor(f"wv{l}", [c.DM, c.D], BF, kind="ExternalInput").ap())
        wo.append(nc.dram_tensor(f"wo{l}", [c.HPC * c.D, c.DM], BF, kind="ExternalInput").ap())
        wg.append(nc.dram_tensor(f"wg{l}", [c.DM, c.FFS], BF, kind="ExternalInput").ap())
        wu.append(nc.dram_tensor(f"wu{l}", [c.DM, c.FFS], BF, kind="ExternalInput").ap())
        wd.append(nc.dram_tensor(f"wd{l}", [c.FFS, c.DM], BF, kind="ExternalInput").ap())
        ktc.append(nc.dram_tensor(f"ktc{l}", [c.HPC, 128, c.CACHE], BF, kind="ExternalInput").ap())
        vc.append(nc.dram_tensor(f"vc{l}", [c.HPC, c.CACHE, c.D], BF, kind="ExternalInput").ap())

    inv_n = 1.0 / c.NCORES
    qk_scale = 1.0 / math.sqrt(c.D)

    with tile.TileContext(nc) as tc:
        with (
            tc.tile_pool(name="consts", bufs=1) as consts,
            tc.tile_pool(name="weights", bufs=1) as wpool,
            tc.tile_pool(name="kv", bufs=1) as kvpool,
            tc.tile_pool(name="acts", bufs=1) as hpool,
            tc.tile_pool(name="xn", bufs=2) as xnpool,
            tc.tile_pool(name="small", bufs=2) as small,
            tc.tile_pool(name="str3", bufs=3) as str3,
            tc.tile_pool(name="psA", bufs=2, space="PSUM") as psA,
            tc.tile_pool(name="psB", bufs=2, space="PSUM") as psB,
            tc.tile_pool(name="dram", bufs=1, space="DRAM") as dram,
        ):
            # ---- constants ----
            cos_sb = consts.tile([128, c.S], BF)
            sin_sb = consts.tile([128, c.S], BF)
            rt_sb = consts.tile([128, 128], BF)
            mask_sb = consts.tile([128, c.NDIAG, c.CS], BF)
            ones_sb = consts.tile([128, 1], BF)
            ones32 = consts.tile([128, 1], F32)
            eps_sb = consts.tile([1, 1], F32)
            tick_sb = consts.tile([1, 16], F32)
            nc.vector.memset(eps_sb[:], c.EPS)
            nc.vector.memset(ones32[:], 1.0)
            nc.vector.memset(tick_sb[:], 1.0)
            nc.sync.dma_start(out=cos_sb[:], in_=cosq[:])
            nc.sync.dma_start(out=sin_sb[:], in_=sinq[:])
            nc.sync.dma_start(out=rt_sb[:], in_=rt[:])
            nc.sync.dma_start(out=mask_sb[:], in_=masks.rearrange("r p n -> p r n"))
            nc.vector.memset(ones_sb[:], 1.0)

            # AR bounce buffers (one pair per layer/phase/chunk)
            arin = {}
            arout = {}
            for l in range(c.L):
                for ph in range(2):
                    for ch in range(c.CH):
                        arin[(l, ph, ch)] = dram.tile(
                            [c.DM, c.CS], BF, tag=f"ari{l}_{ph}_{ch}", name=f"ari{l}_{ph}_{ch}")
                        arout[(l, ph, ch)] = dram.tile(
                            [c.DM, c.CS], BF, tag=f"aro{l}_{ph}_{ch}", name=f"aro{l}_{ph}_{ch}",
                            addr_space="Shared")

            def rms_norm_chunk(h_sb, ch, out_tag):
                """h_sb [128, KD, CS] f32 -> xn [128, KD, CS] bf16 (normalized)."""
                ms_ps = psB.tile([1, c.CS], F32, tag="sum", name=f"ms_{out_tag}")
                for k in range(c.KD):
                    xsq = str3.tile([128, c.CS], BF, tag="xsq", bufs=2, name=f"xsq_{out_tag}_{k}")
                    nc.scalar.activation(out=xsq[:], in_=h_sb[:, k, :], func=AF.Square)
                    nc.tensor.matmul(ms_ps[:], ones_sb[:], xsq[:],
                                     start=(k == 0), stop=(k == c.KD - 1))
                rstd = small.tile([1, c.CS], F32, tag="row", bufs=3, name=f"rstd_{out_tag}")
                nc.scalar.activation(out=rstd[:], in_=ms_ps[:], func=AF.Sqrt,
                                     scale=1.0 / c.DM, bias=eps_sb[:])
                nc.vector.reciprocal(out=rstd[:], in_=rstd[:])
                rb = small.tile([128, c.CS], F32, tag="bcast", bufs=2, name=f"rb_{out_tag}")
                nc.gpsimd.partition_broadcast(rb[:], rstd[:])
                xn = xnpool.tile([128, c.KD, c.CS], BF, tag="xn", name=f"xn_{out_tag}")
                for k in range(c.KD):
                    nc.vector.tensor_tensor(out=xn[:, k, :], in0=h_sb[:, k, :],
                                            in1=rb[:], op=ALU.mult)
                return xn

            def load_h_chunk(src_dram, ch, tag):
                h_sb = hpool.tile([128, c.KD, c.CS], BF, tag="h", name=f"h_{tag}")
                nc.scalar.dma_start(
                    out=h_sb[:],
                    in_=src_dram.rearrange("(k p) n -> p k n", p=128))
                return h_sb

            def rope(p_ps, ch, tag):
                """p_ps [128, CS] f32 PSUM -> bf16 SBUF tile, RoPE applied."""
                p_sb = str3.tile([128, c.CS], BF, tag="prj", bufs=2, name=f"prj_{tag}")
                nc.scalar.copy(out=p_sb[:], in_=p_ps[:])
                rot_ps = psA.tile([128, c.CS], F32, tag="pj", bufs=3, name=f"rot_{tag}")
                nc.tensor.matmul(rot_ps[:], rt_sb[:], p_sb[:], start=True, stop=True)
                cs = cos_sb[:, ch * c.CS:(ch + 1) * c.CS]
                sn = sin_sb[:, ch * c.CS:(ch + 1) * c.CS]
                t1 = small.tile([128, c.CS], BF, tag="t1", name=f"t1_{tag}")
                nc.vector.tensor_tensor(out=t1[:], in0=p_sb[:], in1=cs, op=ALU.mult)
                t2 = small.tile([128, c.CS], BF, tag="t2", name=f"t2_{tag}")
                nc.vector.tensor_tensor(out=t2[:], in0=rot_ps[:], in1=sn, op=ALU.mult)
                out = str3.tile([128, c.CS], BF, tag="rope", bufs=3, name=f"rope_{tag}")
                nc.vector.tensor_tensor(out=out[:], in0=t1[:], in1=t2[:], op=ALU.add)
                return out

            h_src = {ch: h0[:, ch * c.CS:(ch + 1) * c.CS] for ch in range(c.CH)}

            for l in range(c.L):
                # ---- per-layer weights / caches ----
                wq_sb = wpool.tile([128, c.KD, c.HPC * c.D], BF, tag="wq", name=f"wq_sb{l}")
                wk_sb = wpool.tile([128, c.KD, c.D], BF, tag="wk", name=f"wk_sb{l}")
                wv_sb = wpool.tile([128, c.KD, c.D], BF, tag="wv", name=f"wv_sb{l}")
                wo_sb = wpool.tile([128, c.HPC, c.DM], BF, tag="wo", name=f"wo_sb{l}")
                nc.scalar.dma_start(out=wq_sb[:], in_=wq[l].rearrange("(k p) n -> p k n", p=128))
                nc.scalar.dma_start(out=wk_sb[:], in_=wk[l].rearrange("(k p) n -> p k n", p=128))
                nc.scalar.dma_start(out=wv_sb[:], in_=wv[l].rearrange("(k p) n -> p k n", p=128))
                nc.scalar.dma_start(out=wo_sb[:], in_=wo[l].rearrange("(h p) n -> p h n", p=128))
                kc_sb = kvpool.tile([128, c.HPC, c.CACHE], BF, tag="kc", name=f"kc_sb{l}")
                vc_sb = kvpool.tile([128, c.HPC, c.CT, c.D], BF, tag="vc", name=f"vc_sb{l}")
                nc.scalar.dma_start(out=kc_sb[:], in_=ktc[l].rearrange("h p t -> p h t"))
                nc.scalar.dma_start(out=vc_sb[:], in_=vc[l].rearrange("h (t p) d -> p h t d", p=128))

                knew = kvpool.tile([128, c.S], BF, tag="knew", name=f"knew{l}")
                vnew = kvpool.tile([128, c.ST, c.D], BF, tag="vnew", name=f"vnew{l}")

                # =============== PHASE A: attention ===============
                for ch in range(c.CH):
                    h_sb = load_h_chunk(h_src[ch], ch, f"a{l}_{ch}")
                    xn = rms_norm_chunk(h_sb, ch, f"a{l}_{ch}")

                    # qT per head (+rope)
                    qf = []
                    for hh in range(c.HPC):
                        q_ps = psA.tile([128, c.CS], F32, tag="pj", bufs=3, name=f"q_ps{l}_{ch}_{hh}")
                        for k in range(c.KD):
                            nc.tensor.matmul(
                                q_ps[:], wq_sb[:, k, hh * c.D:(hh + 1) * c.D],
                                xn[:, k, :], start=(k == 0), stop=(k == c.KD - 1))
                        qf.append(rope(q_ps, ch, f"q{l}_{ch}_{hh}"))
                    # kT new (+rope) -> knew columns
                    k_ps = psA.tile([128, c.CS], F32, tag="pj", bufs=3, name=f"k_ps{l}_{ch}")
                    for k in range(c.KD):
                        nc.tensor.matmul(k_ps[:], wk_sb[:, k, :], xn[:, k, :],
                                         start=(k == 0), stop=(k == c.KD - 1))
                    kf = rope(k_ps, ch, f"k{l}_{ch}")
                    nc.vector.tensor_copy(out=knew[:, ch * c.CS:(ch + 1) * c.CS], in_=kf[:])
                    # v new -> vnew tiles [s,d]
                    for ss in range(c.CS // 128):
                        st = ch * (c.CS // 128) + ss
                        v_ps = psA.tile([128, c.D], F32, tag="pj", bufs=3, name=f"v_ps{l}_{ch}_{ss}")
                        for k in range(c.KD):
                            nc.tensor.matmul(
                                v_ps[:], xn[:, k, ss * 128:(ss + 1) * 128],
                                wv_sb[:, k, :], start=(k == 0), stop=(k == c.KD - 1))
                        nc.scalar.copy(out=vnew[:, st, :], in_=v_ps[:])

                    # attention per head
                    o_sb = []
                    n_new = ch * c.NDIAG + c.NDIAG  # valid new-key tiles
                    for hh in range(c.HPC):
                        o_ps = psB.tile([128, c.CS], F32, tag="oacc", bufs=1,
                                        name=f"o_ps{l}_{ch}_{hh}")
                        sacc = small.tile([128, c.CS], F32, tag="sacc", bufs=2,
                                          name=f"sacc{l}_{ch}_{hh}")
                        n_tiles = c.CT + n_new
                        for it in range(n_tiles):
                            if it < c.CT:
                                k_lhs = kc_sb[:, hh, it * 128:(it + 1) * 128]
                                v_lhs = vc_sb[:, hh, it, :]
                                diag_r = -1
                            else:
                                j = it - c.CT
                                k_lhs = knew[:, j * 128:(j + 1) * 128]
                                v_lhs = vnew[:, j, :]
                                diag_r = j - ch * c.NDIAG  # >=0 -> masked tile
                            sc_ps = psA.tile([128, c.CS], F32, tag="sc",
                                             name=f"sc{l}_{ch}_{hh}_{it}")
                            nc.tensor.matmul(sc_ps[:], k_lhs, qf[hh][:],
                                             start=True, stop=True)
                            ex = str3.tile([128, c.CS], BF, tag="exp",
                                           name=f"ex{l}_{ch}_{hh}_{it}")
                            nc.scalar.activation(out=ex[:], in_=sc_ps[:],
                                                 func=AF.Exp, scale=qk_scale)
                            if diag_r >= 0:
                                nc.vector.tensor_tensor(
                                    out=ex[:], in0=ex[:], in1=mask_sb[:, diag_r, :],
                                    op=ALU.mult)
                            nc.tensor.matmul(o_ps[:], v_lhs, ex[:],
                                             start=(it == 0), stop=(it == n_tiles - 1))
                            # softmax denominator accumulates on DVE (frees PE)
                            if it == 0:
                                nc.vector.tensor_copy(out=sacc[:], in_=ex[:])
                            else:
                                nc.vector.tensor_tensor(out=sacc[:], in0=sacc[:],
                                                        in1=ex[:], op=ALU.add)
                        s_ps = psB.tile([1, c.CS], F32, tag="sum",
                                        name=f"s_ps{l}_{ch}_{hh}")
                        nc.tensor.matmul(s_ps[:], ones32[:], sacc[:],
                                         start=True, stop=True)
                        # normalize
                        rcp = small.tile([1, c.CS], F32, tag="row", bufs=3, name=f"rcp{l}_{ch}_{hh}")
                        nc.vector.reciprocal(out=rcp[:], in_=s_ps[:])
                        rcb = small.tile([128, c.CS], F32, tag="bcast", bufs=2, name=f"rcb{l}_{ch}_{hh}")
                        nc.gpsimd.partition_broadcast(rcb[:], rcp[:])
                        o_b = str3.tile([128, c.CS], BF, tag="osb", bufs=2, name=f"osb{l}_{ch}_{hh}")
                        nc.vector.tensor_tensor(out=o_b[:], in0=o_ps[:], in1=rcb[:],
                                                op=ALU.mult)
                        o_sb.append(o_b)

                    # Wo (+ h/8 fused) -> AR input
                    for m in range(c.KD):
                        wo_ps = psA.tile([128, c.CS], F32, tag="pj", bufs=3, name=f"wo_ps{l}_{ch}_{m}")
                        for hh in range(c.HPC):
                            nc.tensor.matmul(wo_ps[:], wo_sb[:, hh, m * 128:(m + 1) * 128],
                                             o_sb[hh][:], start=(hh == 0),
                                             stop=(hh == c.HPC - 1))
                        bo = str3.tile([128, c.CS], BF, tag="bo", bufs=2, name=f"bo{l}_{ch}_{m}")
                        nc.vector.scalar_tensor_tensor(
                            out=bo[:], in0=h_sb[:, m, :], scalar=inv_n, in1=wo_ps[:],
                            op0=ALU.mult, op1=ALU.add)
                        nc.sync.dma_start(out=arin[(l, 0, ch)][m * 128:(m + 1) * 128, :],
                                          in_=bo[:])
                    nc.gpsimd.collective_compute(
                        "AllReduce", ALU.add,
                        replica_groups=[list(range(c.NCORES))],
                        ins=[arin[(l, 0, ch)].opt()], outs=[arout[(l, 0, ch)].opt()])

                # =============== PHASE B: MLP ===============
                for ch in range(c.CH):
                    h_sb = load_h_chunk(arout[(l, 0, ch)][:], ch, f"b{l}_{ch}")
                    xn = rms_norm_chunk(h_sb, ch, f"b{l}_{ch}")
                    act = xnpool.tile([128, c.FK, c.CS], BF, tag="act", bufs=1, name=f"act{l}_{ch}")
                    for f in range(c.FK):
                        wg_f = str3.tile([128, c.KD, 128], BF, tag="wgf", bufs=2,
                                         name=f"wgf{l}_{ch}_{f}")
                        wu_f = str3.tile([128, c.KD, 128], BF, tag="wuf", bufs=2,
                                         name=f"wuf{l}_{ch}_{f}")
                        nc.scalar.dma_start(
                            out=wg_f[:], in_=wg[l].rearrange("(k p) n -> p k n", p=128)[
                                :, :, f * 128:(f + 1) * 128])
                        nc.scalar.dma_start(
                            out=wu_f[:], in_=wu[l].rearrange("(k p) n -> p k n", p=128)[
                                :, :, f * 128:(f + 1) * 128])
                        g_ps = psA.tile([128, c.CS], F32, tag="pj", bufs=3, name=f"g_ps{l}_{ch}_{f}")
                        u_ps = psA.tile([128, c.CS], F32, tag="pj", bufs=3, name=f"u_ps{l}_{ch}_{f}")
                        for k in range(c.KD):
                            nc.tensor.matmul(g_ps[:], wg_f[:, k, :], xn[:, k, :],
                                             start=(k == 0), stop=(k == c.KD - 1))
                        for k in range(c.KD):
                            nc.tensor.matmul(u_ps[:], wu_f[:, k, :], xn[:, k, :],
                                             start=(k == 0), stop=(k == c.KD - 1))
                        gs = str3.tile([128, c.CS], BF, tag="gs", bufs=2, name=f"gs{l}_{ch}_{f}")
                        nc.scalar.activation(out=gs[:], in_=g_ps[:], func=AF.Sigmoid)
                        gsg = str3.tile([128, c.CS], BF, tag="gsg", bufs=2, name=f"gsg{l}_{ch}_{f}")
                        nc.vector.tensor_tensor(out=gsg[:], in0=gs[:], in1=g_ps[:],
                                                op=ALU.mult)
                        nc.vector.tensor_tensor(out=act[:, f, :], in0=gsg[:], in1=u_ps[:],
                                                op=ALU.mult)
                    for m in range(c.KD):
                        wd_m = str3.tile([128, c.FK, 128], BF, tag="wdm", bufs=2,
                                         name=f"wdm{l}_{ch}_{m}")
                        nc.scalar.dma_start(
                            out=wd_m[:], in_=wd[l].rearrange("(f p) n -> p f n", p=128)[
                                :, :, m * 128:(m + 1) * 128])
                        d_ps = psA.tile([128, c.CS], F32, tag="pj", bufs=3, name=f"d_ps{l}_{ch}_{m}")
                        for f in range(c.FK):
                            nc.tensor.matmul(d_ps[:], wd_m[:, f, :], act[:, f, :],
                                             start=(f == 0), stop=(f == c.FK - 1))
                        bo = str3.tile([128, c.CS], BF, tag="bo", bufs=2, name=f"bod{l}_{ch}_{m}")
                        nc.vector.scalar_tensor_tensor(
                            out=bo[:], in0=h_sb[:, m, :], scalar=inv_n, in1=d_ps[:],
                            op0=ALU.mult, op1=ALU.add)
                        nc.sync.dma_start(out=arin[(l, 1, ch)][m * 128:(m + 1) * 128, :],
                                          in_=bo[:])
                    nc.gpsimd.collective_compute(
                        "AllReduce", ALU.add,
                        replica_groups=[list(range(c.NCORES))],
                        ins=[arin[(l, 1, ch)].opt()], outs=[arout[(l, 1, ch)].opt()])

                h_src = {ch: arout[(l, 1, ch)][:] for ch in range(c.CH)}

            # =============== final norm + lm_head ===============
            for ch in range(c.CH):
                h_sb = load_h_chunk(h_src[ch], ch, f"f{ch}")
                xn = rms_norm_chunk(h_sb, ch, f"f{ch}")
                nvc = c.VS // c.VCS
                for v in range(nvc):
                    lw = str3.tile([128, c.KD, c.VCS], BF, tag="lw", bufs=2,
                                   name=f"lw{ch}_{v}")
                    nc.scalar.dma_start(
                        out=lw[:], in_=lmw.rearrange("(k p) n -> p k n", p=128)[
                            :, :, v * c.VCS:(v + 1) * c.VCS])
                    for ss in range(c.CS // 128):
                        lm_ps = psA.tile([128, c.VCS], F32, tag="pj", bufs=3, name=f"lm_ps{ch}_{v}_{ss}")
                        for k in range(c.KD):
                            nc.tensor.matmul(lm_ps[:], xn[:, k, ss * 128:(ss + 1) * 128],
                                             lw[:, k, :], start=(k == 0),
                                             stop=(k == c.KD - 1))
                        lo = str3.tile([128, c.VCS], F32, tag="lo", bufs=2, name=f"lo{ch}_{v}_{ss}")
                        nc.scalar.copy(out=lo[:], in_=lm_ps[:])
                        nc.sync.dma_start(
                            out=logits[ch * c.CS + ss * 128: ch * c.CS + (ss + 1) * 128,
                                       v * c.VCS:(v + 1) * c.VCS],
                            in_=lo[:])
            nc.sync.dma_start(out=tick[:], in_=tick_sb[:])

    nc.compile()
    return nc


# ------------------------- host side -------------------------

def rope_tables(c: Cfg, pos):
    inv = 1.0 / (c.ROPE_BASE ** (np.arange(0, c.D, 2, dtype=np.float32) / c.D))
    f = pos[:, None].astype(np.float32) * inv[None, :]
    emb = np.concatenate([f, f], -1)              # [T, D]
    return np.cos(emb), np.sin(emb)


def host_prep(c: Cfg, inputs):
    """inputs: full fp32 arrays keyed as in setup_inputs(). Returns in_maps."""
    ids = np.asarray(inputs["input_ids"]).reshape(-1)
    embed = np.asarray(inputs["embed"], dtype=np.float32)
    h0 = embed[ids]                               # [S, DM] fp32 gather
    h0T = np.ascontiguousarray(h0.T).astype(NPBF)  # [DM, S] bf16

    cos_q, sin_q = rope_tables(c, np.arange(c.CACHE, c.CACHE + c.S))
    cosqT = np.ascontiguousarray(cos_q.T).astype(NPBF)   # [D, S]
    sinqT = np.ascontiguousarray(sin_q.T).astype(NPBF)
    cos_c, sin_c = rope_tables(c, np.arange(c.CACHE))

    # rotation matrix R: rot = R @ x ; lhsT for matmul is R.T
    R = np.zeros((c.D, c.D), np.float32)
    half = c.D // 2
    for i in range(half):
        R[i, i + half] = -1.0
        R[i + half, i] = 1.0
    RT = np.ascontiguousarray(R.T).astype(NPBF)

    # diagonal masks: for r, mask[i, j] = 1 if i + 128*r <= j else 0
    m = np.zeros((c.NDIAG, 128, c.CS), np.float32)
    ii = np.arange(128)[:, None]
    jj = np.arange(c.CS)[None, :]
    for r in range(c.NDIAG):
        m[r] = (ii + 128 * r <= jj)
    masks = m.astype(NPBF)

    ln1 = np.asarray(inputs["ln1"], np.float32)
    ln2 = np.asarray(inputs["ln2"], np.float32)
    fnorm = np.asarray(inputs["final_norm"], np.float32)
    Wq = np.asarray(inputs["Wq"], np.float32)
    Wk = np.asarray(inputs["Wk"], np.float32)
    Wv = np.asarray(inputs["Wv"], np.float32)
    Wo = np.asarray(inputs["Wo"], np.float32)
    Wg = np.asarray(inputs["Wg"], np.float32)
    Wu = np.asarray(inputs["Wu"], np.float32)
    Wd = np.asarray(inputs["Wd"], np.float32)
    lm = np.asarray(inputs["lm_head"], np.float32)
    kc = np.asarray(inputs["k_cache"], np.float32)
    vcache = np.asarray(inputs["v_cache"], np.float32)

    # RoPE the k cache on host (positions 0..CACHE-1), all heads
    rot = np.concatenate([-kc[..., half:], kc[..., :half]], -1)
    kc_roped = kc * cos_c + rot * sin_c           # [L, B, H, CACHE, D]

    in_maps = []
    for core in range(c.NCORES):
        d = {}
        d["h0"] = h0T
        d["cosq"], d["sinq"], d["rt"], d["masks"] = cosqT, sinqT, RT, masks
        d["lmw"] = ((lm * fnorm[:, None])[:, core * c.VS:(core + 1) * c.VS]).astype(NPBF)
        hs = slice(core * c.HPC * c.D, (core + 1) * c.HPC * c.D)
        ks = slice(core * c.D, (core + 1) * c.D)
        fs = slice(core * c.FFS, (core + 1) * c.FFS)
        for l in range(c.L):
            d[f"wq{l}"] = ((Wq[l] * ln1[l][:, None])[:, hs]).astype(NPBF)
            d[f"wk{l}"] = ((Wk[l] * ln1[l][:, None])[:, ks]).astype(NPBF)
            d[f"wv{l}"] = ((Wv[l] * ln1[l][:, None])[:, ks]).astype(NPBF)
            d[f"wo{l}"] = np.ascontiguousarray(Wo[l][hs, :]).astype(NPBF)
            d[f"wg{l}"] = ((Wg[l] * ln2[l][:, None])[:, fs]).astype(NPBF)
            d[f"wu{l}"] = ((Wu[l] * ln2[l][:, None])[:, fs]).astype(NPBF)
            d[f"wd{l}"] = np.ascontiguousarray(Wd[l][fs, :]).astype(NPBF)
            kh = kc_roped[l, 0, core * c.HPC:(core + 1) * c.HPC]   # [HPC, CACHE, D]
            d[f"ktc{l}"] = np.ascontiguousarray(kh.transpose(0, 2, 1)).astype(NPBF)
            d[f"vc{l}"] = np.ascontiguousarray(
                vcache[l, 0, core * c.HPC:(core + 1) * c.HPC]).astype(NPBF)
        in_maps.append(d)
    return in_maps


_NC_CACHE = {}


def get_nc(c: Cfg):
    key = (c.L, c.S, c.DM, c.FF, c.V, c.CS)
    if key not in _NC_CACHE:
        _NC_CACHE[key] = build_nc(c)
    return _NC_CACHE[key]


def kernel(**inputs):
    c = Cfg()
    nc = get_nc(c)
    in_maps = host_prep(c, inputs)
    res = bass_utils.run_bass_kernel_spmd(nc, in_maps, core_ids=list(range(c.NCORES)))
    logits = np.concatenate([res.results[i]["logits"] for i in range(c.NCORES)], axis=1)
    return logits[None].astype(np.float32)


# `kernel(**inputs)` defined above via module-level Cfg/get_nc/host_prep.

